# revision 28
# baseline (speedup 1.0000x reference)
"""Multi-head attention forward on 8 Trainium2 NeuronCores.

Problem: nn.MultiHeadAttention, input [4, 2048, 1024], 16 heads, head_dim 64.

Sharding: core = (batch b, head-group g) with b = core // 2, g = core % 2.
Each core computes attention for its 8 heads plus the row-parallel slice of
the output projection, writing two bf16 partials (head-pairs 0+1 and 2+3);
the host sums the four partials per batch and adds folded biases
(bo + bv @ woT).

All matmuls are bf16 with fp32 PSUM accumulation.  Layouts:
  - x kept transposed (xT [E, S], one SBUF tile [128, KT, S]); Q/K produced
    transposed (qt/kt [feat, tok], wq prescaled by 1/sqrt(d), bias added on
    the DVE drain); V produced in normal orientation into V' = [V_h | 1]
    per key tile (ones column makes the AV matmul emit softmax denominators
    for free).
  - scores per head: S^T[keys, q] = K^T.T @ Q^T (64-partition contraction,
    N=512), exp() on the Activation engine into bf16 ring tiles.
  - AV transposed: out[q, 65] = exp_tile.T @ V' (M=128 queries, N=65), one
    sequential PSUM chain per query tile, two chains packed per bank.
  - normalization: per-partition reciprocal + scale into stg[q, pair-heads];
    a deferred PE transpose yields AO^T[hd, q]; the out-projection
    accumulates two head-pairs per PSUM tile.

Slabs (scores+exp per (head, 1024-query chunk)) are emitted head-major with
AV lagging ~2 slabs (exp ring of 40 tiles) so the Activation engine — the
densest engine at ~267 us — stays saturated; projections, V, transposes and
out-projections fill the PE slack between slabs.  All DMAs ride the SP
hardware-DGE queue: the Activation queue must stay DMA-free or DMA
issue time stalls the exp pipeline.
"""

import numpy as np
import ml_dtypes

import concourse.bass as bass
import concourse.mybir as mybir
import concourse.tile as tile
from concourse import bacc
from concourse.bass_utils import run_bass_kernel_spmd

B = 4
S = 2048
E = 1024
H = 16
D = 64
N_CORES = 8
HPC = H // 2            # 8 heads per core
DH = HPC * D            # 512 per-core qkv slice width
FT = DH // 128          # 4 feature tiles / head pairs
KT = E // 128           # 8 contraction tiles over embed dim
TT = S // 128           # 16 token/key tiles
F32 = mybir.dt.float32
BF16 = mybir.dt.bfloat16
EXPF = mybir.ActivationFunctionType.Exp
COPYF = mybir.ActivationFunctionType.Copy

_CACHE = {}

TRACE = False
LAST_RESULTS = None

EXP_RING = 43           # exp tiles [128, 1024] bf16 in flight


def _build_program():
    nc = bacc.Bacc("TRN2", target_bir_lowering=False, debug=False)

    xT_d = nc.dram_tensor("xT", [E, S], BF16, kind="ExternalInput")
    wqT_d = nc.dram_tensor("wqT", [E, DH], BF16, kind="ExternalInput")
    wkT_d = nc.dram_tensor("wkT", [E, DH], BF16, kind="ExternalInput")
    wvT_d = nc.dram_tensor("wvT", [E, DH], BF16, kind="ExternalInput")
    woT_d = nc.dram_tensor("woT", [DH, E], BF16, kind="ExternalInput")
    bq_d = nc.dram_tensor("bq", [128, FT], F32, kind="ExternalInput")
    bk_d = nc.dram_tensor("bk", [128, FT], F32, kind="ExternalInput")
    ident_d = nc.dram_tensor("ident", [128, 128], BF16, kind="ExternalInput")
    ones_d = nc.dram_tensor("ones", [128, TT * HPC], BF16,
                            kind="ExternalInput")
    yp_d = [nc.dram_tensor(f"y{p}", [S, E], BF16, kind="ExternalOutput")
            for p in range(2)]

    with tile.TileContext(nc) as tc:
        with tc.tile_pool(name="persist", bufs=1) as pp:
            ident = pp.tile([128, 128], BF16, name="ident")
            bq = pp.tile([128, FT], F32, name="bq")
            bk = pp.tile([128, FT], F32, name="bk")
            vs = pp.tile([128, TT, HPC, 65], BF16, name="vs")
            qt = [pp.tile([128, S], BF16, name=f"qt{f}") for f in range(FT)]
            kt = [pp.tile([128, S], BF16, name=f"kt{f}") for f in range(FT)]

            with (
                tc.tile_pool(name="psS", bufs=2, space="PSUM") as psS,
                tc.tile_pool(name="psAV", bufs=1, space="PSUM") as psAV,
                tc.tile_pool(name="psT", bufs=1, space="PSUM") as psT,
                tc.tile_pool(name="expp", bufs=EXP_RING) as ep,
                tc.tile_pool(name="stgp", bufs=2) as stgp,
                tc.tile_pool(name="aotpp", bufs=2) as aotpool,
                tc.tile_pool(name="small", bufs=4) as sp,
            ):
                stg_tiles = {}
                aotp = {}
                # persistent AV accumulator: two alternating sequential
                # chains packed into a single PSUM bank
                pavt = psAV.tile([128, 2, 65], F32, name="pavt", tag="pav")

                def slab_part(h, qc, ts, tiles):
                    f, hr = h // 2, (h % 2) * 64
                    for t in ts:
                        ps = psS.tile([128, 1024], F32, name="ps", tag="sc")
                        for j in range(2):
                            nc.tensor.matmul(
                                ps[:, j * 512:(j + 1) * 512],
                                kt[f][hr:hr + 64, t * 128:(t + 1) * 128],
                                qt[f][hr:hr + 64,
                                      qc * 1024 + j * 512:
                                      qc * 1024 + (j + 1) * 512],
                                start=True, stop=True)
                        e = ep.tile([128, 1024], BF16, name="e", tag="exp")
                        nc.scalar.activation(e[:], ps[:], EXPF)
                        tiles.append(e)

                st = {}

                def sa(i):
                    st[i] = []
                    slab_part(i // 2, i % 2, range(0, 8), st[i])

                def sb(i):
                    slab_part(i // 2, i % 2, range(8, 16), st[i])

                def A(i, qlo=0, qhi=8):
                    h, qc = i // 2, i % 2
                    p, hc = h // 2, (h % 2) * 64
                    if p not in stg_tiles:
                        stg_tiles[p] = stgp.tile([128, TT, 128], BF16,
                                                 name="stg", tag="stg")
                    tiles = st.pop(i) if qhi == 8 else st[i]
                    for qi in range(qlo, qhi):
                        pav = pavt[:, qi % 2, :]
                        for t in range(TT):
                            nc.tensor.matmul(
                                pav, tiles[t][:, qi * 128:(qi + 1) * 128],
                                vs[:, t, h, :],
                                start=(t == 0), stop=(t == TT - 1))
                        qtg = qc * 8 + qi
                        rc = sp.tile([128, 1], F32, name="rc", tag="rc")
                        nc.vector.reciprocal(rc[:], pav[:, 64:65])
                        nc.vector.tensor_scalar_mul(
                            stg_tiles[p][:, qtg, hc:hc + 64],
                            pav[:, 0:64], rc[:])

                def transpose_pair(p, qlo=0, qhi=TT):
                    if p not in aotp:
                        aotp[p] = aotpool.tile([128, S], BF16, name="aot",
                                               tag="aot")
                    for q in range(qlo, qhi):
                        pt = psT.tile([128, 2, 128], BF16, name="pt",
                                      tag="pt")
                        sl = pt[:, q % 2, :]
                        nc.tensor.transpose(sl, stg_tiles[p][:, q, :],
                                            ident[:])
                        nc.vector.tensor_copy(
                            aotp[p][:, q * 128:(q + 1) * 128], sl)

                # ---- era 1: projections + V + slabs S0..S12 --------------
                with (
                    tc.tile_pool(name="wqk", bufs=1) as wp,
                    tc.tile_pool(name="xp", bufs=1) as xp,
                    tc.tile_pool(name="psP", bufs=2, space="PSUM") as psP,
                ):
                    wk = wp.tile([128, KT, DH], BF16, name="wk")
                    wq = wp.tile([128, KT, DH], BF16, name="wq")
                    xt = xp.tile([128, KT, S], BF16, name="xt")
                    wvp_cm = tc.tile_pool(name="wvp", bufs=1)
                    wvp = wvp_cm.__enter__()
                    wv = wvp.tile([128, KT, DH], BF16, name="wv")

                    kp = lambda d: d.ap().rearrange("(k p) f -> p k f", p=128)
                    # single SP HWDGE queue; transfers serialize, so order
                    # equals arrival order.  ident rides first and feeds
                    # PE-warmup dummy transposes so the startup projection
                    # chains run at the full 2.4GHz p-state.
                    nc.sync.dma_start(ident[:], ident_d.ap())
                    nc.sync.dma_start(wk[:, :, 0:128], kp(wkT_d)[:, :, 0:128])
                    nc.sync.dma_start(xt[:, 0:4, 0:512],
                                      kp(xT_d)[:, 0:4, 0:512])
                    nc.sync.dma_start(xt[:, 4:8, 0:512],
                                      kp(xT_d)[:, 4:8, 0:512])
                    nc.sync.dma_start(wq[:, :, 0:128], kp(wqT_d)[:, :, 0:128])
                    nc.sync.dma_start(bq[:], bq_d.ap())
                    nc.sync.dma_start(bk[:], bk_d.ap())
                    nc.sync.dma_start(xt[:, :, 512:1024],
                                      kp(xT_d)[:, :, 512:1024])
                    nc.sync.dma_start(xt[:, :, 1024:1536],
                                      kp(xT_d)[:, :, 1024:1536])
                    nc.sync.dma_start(xt[:, :, 1536:2048],
                                      kp(xT_d)[:, :, 1536:2048])
                    nc.sync.dma_start(wv[:], kp(wvT_d))
                    nc.sync.dma_start(wk[:, :, 128:DH],
                                      kp(wkT_d)[:, :, 128:DH])
                    nc.sync.dma_start(wq[:, :, 128:DH],
                                      kp(wqT_d)[:, :, 128:DH])
                    nc.sync.dma_start(
                        vs[:, :, :, 64],
                        ones_d.ap().rearrange("p (t h) -> p t h", t=TT))

                    def proj_qk(dst, w, f, bias, chunks):
                        for c in chunks:
                            pq = psP.tile([128, 512], F32, name="pq",
                                          tag="pq")
                            for k in range(KT):
                                nc.tensor.matmul(
                                    pq[:], w[:, k, f * 128:(f + 1) * 128],
                                    xt[:, k, c * 512:(c + 1) * 512],
                                    start=(k == 0), stop=(k == KT - 1))
                            nc.vector.tensor_scalar_add(
                                dst[f][:, c * 512:(c + 1) * 512], pq[:],
                                bias[:, f:f + 1])

                    def proj_v(t_lo, t_hi):
                        for t in range(t_lo, t_hi):
                            pv = psP.tile([128, 512], F32, name="pv",
                                          tag="pq")
                            for k in range(KT):
                                nc.tensor.matmul(
                                    pv[:], xt[:, k, t * 128:(t + 1) * 128],
                                    wv[:, k, :],
                                    start=(k == 0), stop=(k == KT - 1))
                            nc.vector.tensor_copy(
                                vs[:, t, :, 0:64],
                                pv[:].rearrange("p (h d) -> p h d", h=HPC))

                    wu = psT.tile([128, 2, 128], BF16, name="pt", tag="pt")
                    for _ in range(26):
                        nc.tensor.transpose(wu[:, 0, :], ident[:], ident[:])

                    # fine-grained start: consume x chunks as they land.
                    # deps: slab i (head i//2, qchunk i%2, f=i//4) needs
                    # K_f complete + Q_f c01 before sa(4f), Q_f c23 before
                    # sa(4f+1); AV(k) needs all of V; exp-ring(42) needs
                    # A(k-2) emitted before slab k's tile 10.
                    proj_qk(kt, wk, 0, bk, (0,))
                    proj_qk(qt, wq, 0, bq, (0,))
                    proj_qk(kt, wk, 0, bk, (1,))
                    proj_qk(qt, wq, 0, bq, (1,))
                    sa(0)
                    proj_qk(kt, wk, 0, bk, (2,))
                    proj_qk(qt, wq, 0, bq, (2,))
                    slab_part(0, 0, range(8, 12), st[0])
                    proj_qk(kt, wk, 0, bk, (3,))
                    proj_qk(qt, wq, 0, bq, (3,))
                    slab_part(0, 0, range(12, 16), st[0])
                    proj_v(0, 4)
                    sa(1)
                    proj_v(4, 8)
                    sb(1)
                    proj_v(8, 12)
                    sa(2)
                    slab_part(1, 0, range(8, 11), st[2])
                    proj_v(12, 16)
                    wvp_cm.__exit__(None, None, None)
                    A(0)
                    slab_part(1, 0, range(11, 16), st[2])
                    proj_qk(kt, wk, 1, bk, (0, 1))
                    sa(3)
                    A(1)
                    proj_qk(kt, wk, 1, bk, (2, 3))
                    sb(3)
                    proj_qk(qt, wq, 1, bq, (0, 1))
                    sa(4)
                    A(2)
                    proj_qk(qt, wq, 1, bq, (2, 3))
                    sb(4)
                    proj_qk(kt, wk, 2, bk, (0, 1))
                    sa(5)
                    A(3)
                    proj_qk(kt, wk, 2, bk, (2, 3))
                    sb(5)
                    proj_qk(qt, wq, 2, bq, (0, 1))
                    sa(6)
                    A(4)
                    proj_qk(qt, wq, 2, bq, (2, 3))
                    sb(6)
                    proj_qk(kt, wk, 3, bk, (0, 1))
                    sa(7)
                    A(5)
                    proj_qk(kt, wk, 3, bk, (2, 3))
                    sb(7)
                    proj_qk(qt, wq, 3, bq, (0, 1))
                    sa(8)
                    A(6)
                    proj_qk(qt, wq, 3, bq, (2, 3))
                    sb(8)
                    transpose_pair(0)
                    sa(9)
                    A(7)
                    sb(9)
                    transpose_pair(1)
                    sa(10)
                    A(8)
                    sb(10)
                    sa(11)
                    A(9)
                    sb(11)
                    sa(12)
                    A(10)
                    sb(12)

                # ---- era 2: last slabs + combined out-projections --------
                with (
                    tc.tile_pool(name="wop", bufs=1) as wop,
                    tc.tile_pool(name="ysp", bufs=4) as yp,
                    tc.tile_pool(name="psY", bufs=2, space="PSUM") as psY,
                ):
                    wo = wop.tile([128, FT, E], BF16, name="wo")
                    nc.sync.dma_start(
                        wo[:],
                        woT_d.ap().rearrange("(p q) f -> q p f", q=128))

                    def out_proj(g, t_lo, t_hi, use_act=False):
                        """partial y_g = sum of two pairs' AO^T @ Wo."""
                        for t in range(t_lo, t_hi, 2):
                            ys = yp.tile([128, 2, E], BF16, name="ys",
                                         tag="ys")
                            for tt in (t, t + 1):
                                for j in range(2):
                                    py = psY.tile([128, 512], F32,
                                                  name="py", tag="py")
                                    for p in (2 * g, 2 * g + 1):
                                        nc.tensor.matmul(
                                            py[:],
                                            aotp[p][:,
                                                    tt * 128:(tt + 1) * 128],
                                            wo[:, p,
                                               j * 512:(j + 1) * 512],
                                            start=(p == 2 * g),
                                            stop=(p == 2 * g + 1))
                                    dst = ys[:, tt - t,
                                             j * 512:(j + 1) * 512]
                                    if use_act and (tt + j) % 2 == 0:
                                        nc.scalar.activation(dst, py[:],
                                                             COPYF)
                                    else:
                                        nc.vector.tensor_copy(dst, py[:])
                            nc.sync.dma_start(
                                yp_d[g].ap()[t * 128:(t + 2) * 128, :]
                                .rearrange("(j p) f -> p j f", p=128),
                                ys[:])

                    sa(13)
                    A(11)
                    sb(13)
                    out_proj(0, 0, 8)
                    sa(14)
                    A(12)
                    sb(14)
                    out_proj(0, 8, 16)
                    transpose_pair(2)
                    sa(15)
                    A(13)
                    sb(15)
                    A(14)
                    transpose_pair(3, 0, 8)      # qc0 rows ready after A13
                    out_proj(1, 0, 8)            # DVE copies: ACT still on exps
                    for _q in range(0, 8, 2):
                        A(15, _q, _q + 2)
                        transpose_pair(3, 8 + _q, 8 + _q + 2)
                        out_proj(1, 8 + _q, 8 + _q + 2, use_act=True)

    nc.compile()
    return nc


def kernel(input_tensor, wq, bq, wk, bk_, wv, bv, wo, bo):
    global LAST_RESULTS
    if "nc" not in _CACHE:
        _CACHE["nc"] = _build_program()
    nc = _CACHE["nc"]

    BF = ml_dtypes.bfloat16
    x = np.asarray(input_tensor, dtype=np.float32)
    scale = np.float32(1.0 / np.sqrt(np.float32(D)))

    wqT = (np.asarray(wq, np.float32).T * scale).astype(BF)
    wkT = np.asarray(wk, np.float32).T.astype(BF)
    wvT = np.asarray(wv, np.float32).T.astype(BF)
    woT = np.ascontiguousarray(np.asarray(wo, np.float32).T)
    bqs = np.asarray(bq, np.float32) * scale
    bkf = np.asarray(bk_, np.float32)

    ident = np.eye(128, dtype=BF)
    ones = np.ones((128, TT * HPC), dtype=BF)

    in_maps = []
    for core in range(N_CORES):
        b, g = divmod(core, 2)
        hs = slice(g * DH, (g + 1) * DH)
        in_maps.append({
            "xT": np.ascontiguousarray(x[b].T).astype(BF),
            "wqT": np.ascontiguousarray(wqT[:, hs]),
            "wkT": np.ascontiguousarray(wkT[:, hs]),
            "wvT": np.ascontiguousarray(wvT[:, hs]),
            "woT": woT[hs, :].astype(BF),
            "bq": np.ascontiguousarray(bqs[hs].reshape(FT, 128).T),
            "bk": np.ascontiguousarray(bkf[hs].reshape(FT, 128).T),
            "ident": ident,
            "ones": ones,
        })

    bias_full = (np.asarray(bo, np.float32)
                 + np.asarray(bv, np.float32) @ np.asarray(wo, np.float32).T
                 ).astype(np.float32)

    def _run():
        global LAST_RESULTS
        res = run_bass_kernel_spmd(nc, in_maps,
                                   core_ids=list(range(N_CORES)),
                                   trace=TRACE)
        LAST_RESULTS = res
        # unshard: sum the four bf16 partials (2 cores x 2 pair-groups)
        # per batch, add folded biases
        y = np.empty((B, S, E), np.float32)
        for b in range(B):
            acc = bias_full[None, :].repeat(S, axis=0)
            for core in (2 * b, 2 * b + 1):
                for g in range(2):
                    acc = acc + res.results[core][f"y{g}"].astype(np.float32)
            y[b] = acc
        return y

    def _expected_row0():
        # host-computed reference for y[0, 0, :]: cheap guard against a
        # flaky first device execution after terminal boot
        xb = x[0]
        q0 = (xb[0:1] @ np.asarray(wq, np.float32).T
              + np.asarray(bq, np.float32)) / np.sqrt(np.float32(D))
        kf = xb @ np.asarray(wk, np.float32).T + np.asarray(bk_, np.float32)
        vf = xb @ np.asarray(wv, np.float32).T + np.asarray(bv, np.float32)
        wof = np.asarray(wo, np.float32)
        out = np.asarray(bo, np.float32).copy()
        for h in range(H):
            s = q0[0, h * D:(h + 1) * D] @ kf[:, h * D:(h + 1) * D].T
            e = np.exp(s - s.max())
            ao = (e @ vf[:, h * D:(h + 1) * D]) / e.sum()
            out += ao @ wof.T[h * D:(h + 1) * D]
        return out

    y = _run()
    row = _expected_row0()
    for _retry in range(2):
        rel = np.abs(y[0, 0] - row).max() / (np.abs(row).max() + 1e-9)
        if rel < 0.2:
            break
        y = _run()
    return y

